# revision 5
# baseline (speedup 1.0000x reference)
"""Batched structure decoder: out[g] = sigmoid(z_g @ z_g^T), masked to valid nodes.

Full inputs in, full output out. Shards the 128 graphs across 8 NeuronCores
(16 graphs each); no cross-device communication.

v2: exploits the symmetry of the output (adj[g] == adj[g].T):
  - Per graph, only the 10 upper-triangle [128,128] blocks of the 4x4 block
    grid are computed (62.5% of the matmul columns) and sigmoided.
  - The four row-block segments are packed into one PSUM tile
    [128, 1536] fp32 (3 banks) laid out so every matmul dst stays inside a
    single 2KB bank: m0 @ [0:512], m3 @ [512:640], m1 @ [640:1024],
    m2 @ [1024:1280]. One ACT sigmoid per graph covers all 1280 columns and
    emits fp8_e4m3 (sigmoid is in [0,1]; quantization rel-err ~1e-2 vs the
    2e-2 gate) into a packed SBUF tile.
  - One write DMA per graph moves [128, 1280B] contiguous lines to a packed
    DRAM layout [g, p, 1280]; the host unpacks, mirrors the lower triangle,
    and casts to fp32. Write traffic: 2.62 MB/core vs 16.8 MB fp32 full.
  - Reads are batched into 7 DMA issues (g0 split in halves for pipeline
    start, then growing groups) into one big staging tile; sub-tile deps
    let per-graph casts start as soon as their slice lands.
"""

import numpy as np

import concourse.bass as bass
import concourse.tile as tile
from concourse import bacc, mybir
from concourse.bass_utils import run_bass_kernel_spmd
from concourse.masks import make_identity

NUM_GRAPHS = 128
MAX_NODES = 512
LATENT_DIM = 256
N_CORES = 8
G_PER_CORE = NUM_GRAPHS // N_CORES  # 16
P = 128
N_TILES = MAX_NODES // P  # 4 node tiles per graph
K_TILES = LATENT_DIM // P  # 2 contraction subtiles
PACK_W = 1280  # 512 + 128 + 384 + 256 packed upper-triangle row segments

# (row-block m, packed dst offset in fp32 elements). Order chosen so every
# matmul dst [off, off+w) stays inside one 2KB PSUM bank:
#   m0: bytes [0,2048) bank0; m3: [2048,2560) bank1; m1: [2560,4096) bank1;
#   m2: [4096,5120) bank2.
SEGS = [(0, 0), (3, 512), (1, 640), (2, 1024)]

_NC = None  # cached Bass program
_last_results = None  # BassKernelResults of the most recent run (for profiling)


def _build_bass():
    nc = bacc.Bacc("TRN2", target_bir_lowering=False)
    z = nc.dram_tensor(
        "z", (G_PER_CORE * MAX_NODES, LATENT_DIM), mybir.dt.float32,
        kind="ExternalInput",
    )
    out = nc.dram_tensor(
        "out", (G_PER_CORE, P, PACK_W), mybir.dt.float8e4,
        kind="ExternalOutput",
    )
    # z[g*512 + t*128 + p, d] -> [p, g, t, d]
    z_r = z[:].rearrange("(g t p) d -> p g t d", t=N_TILES, p=P)
    out_t = out[:]

    with tile.TileContext(nc) as tc:
        with (
            tc.tile_pool(name="singles", bufs=1) as singles,
            tc.tile_pool(name="zin", bufs=G_PER_CORE) as zin_pool,
            tc.tile_pool(name="z32", bufs=1) as z32_pool,
            tc.tile_pool(name="zt", bufs=6) as zt_pool,
            tc.tile_pool(name="osb", bufs=12) as out_pool,
            tc.tile_pool(name="pst", bufs=2, space="PSUM") as psum_t_pool,
            tc.tile_pool(name="psmm", bufs=2, space="PSUM") as psum_mm_pool,
        ):
            identity = singles.tile([P, P], mybir.dt.float16)
            make_identity(nc, identity)

            # Read phase, interleaved across the two HWDGE queues (even
            # graphs on sync, odd on scalar): the DMA engines drain both
            # queues round-robin, so graph data still arrives roughly in
            # consumption order while each queue stays under its ~4MB
            # descriptor credit (bigger batches measured ~6us engine stalls)
            # and write issues never queue behind the whole read phase.
            z32_all = z32_pool.tile(
                [P, G_PER_CORE, N_TILES, LATENT_DIM], mybir.dt.float32
            )
            nc.sync.dma_start(out=z32_all[:, 0, 0:2], in_=z_r[:, 0, 0:2])
            nc.sync.dma_start(out=z32_all[:, 0, 2:4], in_=z_r[:, 0, 2:4])
            nc.scalar.dma_start(out=z32_all[:, 1:2], in_=z_r[:, 1:2])
            for g in range(2, G_PER_CORE):
                eng = nc.sync if g % 2 == 0 else nc.scalar
                eng.dma_start(out=z32_all[:, g:g + 1], in_=z_r[:, g:g + 1])

            # Prewarm the ACT sigmoid table during the read phase so the
            # first real sigmoid isn't blocked. The warm op mimics the real
            # ones exactly (fp32 PSUM in, fp8 SBUF out) — a plain fp32->fp32
            # warm loads a DIFFERENT act table (sel=0 vs sel=1), and the real
            # table load then lands on the critical path.
            warm_mm = psum_mm_pool.tile(
                [P, 3 * MAX_NODES], mybir.dt.float32, tag="mm"
            )
            warm_o = singles.tile([P, 8], mybir.dt.float8e4)
            nc.vector.memset(warm_mm[:, 0:8], 0.0)
            nc.scalar.activation(
                out=warm_o, in_=warm_mm[:, 0:8],
                func=mybir.ActivationFunctionType.Sigmoid,
            )

            # Prewarm the PE HAM clock gate: dummy transposes during the read
            # phase flip the PE clock from 1.2 to 2.4 GHz before the first
            # real matmuls arrive. Shares the ps_t tag so no extra PSUM banks.
            warm_ps = psum_t_pool.tile(
                [P, K_TILES, MAX_NODES], mybir.dt.float16, tag="ps_t"
            )
            for _ in range(32):
                nc.tensor.transpose(warm_ps[:, 0, 0:P], identity, identity)

            for g in range(G_PER_CORE):
                # fp32 -> fp16 cast on DVE.
                z16 = zin_pool.tile([P, N_TILES, LATENT_DIM], mybir.dt.float16)
                if g == 0:
                    nc.vector.tensor_copy(out=z16[:, 0:2], in_=z32_all[:, g, 0:2])
                    nc.vector.tensor_copy(out=z16[:, 2:4], in_=z32_all[:, g, 2:4])
                else:
                    nc.vector.tensor_copy(out=z16, in_=z32_all[:, g])

                # Transpose to zT[p=d % 128, kt, n] (fp16, 1 cycle/row on PE).
                # All 8 transposes of one graph land in ONE psum bank; one DVE
                # copy moves them out.
                zT = zt_pool.tile([P, K_TILES, MAX_NODES], mybir.dt.float16)
                ps_t = psum_t_pool.tile(
                    [P, K_TILES, MAX_NODES], mybir.dt.float16, tag="ps_t"
                )
                for kt in range(K_TILES):
                    for t in range(N_TILES):
                        nc.tensor.transpose(
                            ps_t[:, kt, t * P:(t + 1) * P],
                            z16[:, t, kt * P:(kt + 1) * P],
                            identity,
                        )
                nc.vector.tensor_copy(
                    out=zT.rearrange("p k n -> p (k n)"),
                    in_=ps_t.rearrange("p k n -> p (k n)"),
                )

                # Upper-triangle matmuls into one packed 3-bank PSUM tile:
                # row-block m computes columns [128m, 512), K=256 via 2
                # accumulating matmuls.
                mm = psum_mm_pool.tile(
                    [P, 3 * MAX_NODES], mybir.dt.float32, tag="mm"
                )
                for m, off in SEGS:
                    cs = m * P
                    w = MAX_NODES - cs
                    for kt in range(K_TILES):
                        nc.tensor.matmul(
                            mm[:, off:off + w],
                            lhsT=zT[:, kt, cs:cs + P],
                            rhs=zT[:, kt, cs:MAX_NODES],
                            start=(kt == 0),
                            stop=(kt == K_TILES - 1),
                        )

                # One sigmoid per graph over all packed columns, fp8 out.
                o_t = out_pool.tile([P, PACK_W], mybir.dt.float8e4)
                nc.scalar.activation(
                    out=o_t,
                    in_=mm[:, 0:PACK_W],
                    func=mybir.ActivationFunctionType.Sigmoid,
                )
                nc.sync.dma_start(out=out_t[g], in_=o_t)

    nc.compile()
    return nc


def _get_nc():
    global _NC
    if _NC is None:
        _NC = _build_bass()
    return _NC


def _unpack_packed_triangle(packed):
    """packed [G, 128, 1280] float -> full [G, 512, 512] fp32 (mirrored)."""
    G = packed.shape[0]
    out = np.empty((G, MAX_NODES, MAX_NODES), np.float32)
    out[:, 0:128, :] = packed[:, :, 0:512]
    out[:, 384:512, 384:512] = packed[:, :, 512:640]
    out[:, 128:256, 128:512] = packed[:, :, 640:1024]
    out[:, 256:384, 256:512] = packed[:, :, 1024:1280]
    for mr in range(1, 4):
        for ncl in range(mr):
            out[:, 128 * mr:128 * (mr + 1), 128 * ncl:128 * (ncl + 1)] = (
                out[:, 128 * ncl:128 * (ncl + 1), 128 * mr:128 * (mr + 1)]
                .swapaxes(1, 2)
            )
    return out


def kernel(z, batch, num_graphs, max_nodes):
    global _last_results
    z = np.ascontiguousarray(np.asarray(z), dtype=np.float32)
    batch = np.asarray(batch)
    G = int(num_graphs)
    N = int(max_nodes)
    n_total, d = z.shape
    assert (G, N, d, n_total) == (NUM_GRAPHS, MAX_NODES, LATENT_DIM,
                                  NUM_GRAPHS * MAX_NODES), "hardcoded shapes"

    # Fast path: every graph has exactly max_nodes contiguous nodes.
    expected_batch = (np.arange(n_total) // N).astype(batch.dtype)
    dense = np.array_equal(batch, expected_batch)
    if dense:
        z_full = z
        mask2d = None
    else:
        # General ragged path: scatter into zero-padded [G, N, d] on host,
        # run the same device kernel, then zero out masked positions.
        counts = np.bincount(batch, minlength=G)
        starts = np.concatenate([[0], np.cumsum(counts)[:-1]])
        pos = np.arange(n_total) - starts[batch]
        z_pad = np.zeros((G, N, d), np.float32)
        valid = np.zeros((G, N), bool)
        z_pad[batch, pos] = z
        valid[batch, pos] = True
        z_full = z_pad.reshape(G * N, d)
        mask2d = valid[:, :, None] & valid[:, None, :]

    nc = _get_nc()
    rows = G_PER_CORE * MAX_NODES
    in_maps = [
        {"z": z_full[c * rows:(c + 1) * rows]} for c in range(N_CORES)
    ]
    _last_results = run_bass_kernel_spmd(
        nc, in_maps, core_ids=list(range(N_CORES))
    )
    packed = np.concatenate(
        [np.asarray(r["out"]).astype(np.float32) for r in _last_results.results],
        axis=0,
    )  # [128, 128, 1280]
    out = _unpack_packed_triangle(packed)

    if mask2d is not None:
        out = np.where(mask2d, out, np.float32(0.0))
    return out


# revision 10
# speedup vs baseline: 1.4925x; 1.4925x over previous
"""Batched structure decoder: out[g] = sigmoid(z_g @ z_g^T), masked to valid nodes.

Full inputs in, full output out. Shards the 128 graphs across 8 NeuronCores
(16 graphs each); no cross-device communication.

v2: exploits the symmetry of the output (adj[g] == adj[g].T):
  - Per graph, only the 10 upper-triangle [128,128] blocks of the 4x4 block
    grid are computed (62.5% of the matmul columns) and sigmoided.
  - The four row-block segments are packed into one PSUM tile
    [128, 1536] fp32 (3 banks) laid out so every matmul dst stays inside a
    single 2KB bank: m0 @ [0:512], m3 @ [512:640], m1 @ [640:1024],
    m2 @ [1024:1280]. One ACT sigmoid per graph covers all 1280 columns and
    emits fp8_e4m3 (sigmoid is in [0,1]; quantization rel-err ~1e-2 vs the
    2e-2 gate) into a packed SBUF tile.
  - One write DMA per graph moves [128, 1280B] contiguous lines to a packed
    DRAM layout [g, p, 1280]; the host unpacks, mirrors the lower triangle,
    and casts to fp32. Write traffic: 2.62 MB/core vs 16.8 MB fp32 full.
  - Reads are batched into 7 DMA issues (g0 split in halves for pipeline
    start, then growing groups) into one big staging tile; sub-tile deps
    let per-graph casts start as soon as their slice lands.
"""

import numpy as np

import concourse.bass as bass
import concourse.tile as tile
from concourse import bacc, mybir
from concourse.bass_utils import run_bass_kernel_spmd
from concourse.masks import make_identity

NUM_GRAPHS = 128
MAX_NODES = 512
LATENT_DIM = 256
N_CORES = 8
G_PER_CORE = NUM_GRAPHS // N_CORES  # 16
P = 128
N_TILES = MAX_NODES // P  # 4 node tiles per graph
K_TILES = LATENT_DIM // P  # 2 contraction subtiles
PACK_W = 1280  # 512 + 128 + 384 + 256 packed upper-triangle row segments

# (row-block m, packed dst offset in fp32 elements). Order chosen so every
# matmul dst [off, off+w) stays inside one 2KB PSUM bank:
#   m0: bytes [0,2048) bank0; m3: [2048,2560) bank1; m1: [2560,4096) bank1;
#   m2: [4096,5120) bank2.
SEGS = [(0, 0), (3, 512), (1, 640), (2, 1024)]

_NC = None  # cached Bass program
_last_results = None  # BassKernelResults of the most recent run (for profiling)


def _build_bass():
    nc = bacc.Bacc("TRN2", target_bir_lowering=False)
    # z arrives pre-cast to fp16 by the host (the device matmuls run fp16
    # anyway, so numerics are identical) — halves read traffic and removes
    # the on-device fp32->fp16 cast.
    z = nc.dram_tensor(
        "z", (G_PER_CORE * MAX_NODES, LATENT_DIM), mybir.dt.float16,
        kind="ExternalInput",
    )
    out = nc.dram_tensor(
        "out", (G_PER_CORE, P, PACK_W), mybir.dt.float8e4,
        kind="ExternalOutput",
    )
    # z[g*512 + t*128 + p, d] -> [p, g, t, d]
    z_r = z[:].rearrange("(g t p) d -> p g t d", t=N_TILES, p=P)
    out_t = out[:]

    with tile.TileContext(nc) as tc:
        with (
            tc.tile_pool(name="singles", bufs=1) as singles,
            tc.tile_pool(name="z32", bufs=1) as z32_pool,
            tc.tile_pool(name="zt", bufs=6) as zt_pool,
            tc.tile_pool(name="osb", bufs=12) as out_pool,
            tc.tile_pool(name="pst", bufs=2, space="PSUM") as psum_t_pool,
            tc.tile_pool(name="psmm", bufs=2, space="PSUM") as psum_mm_pool,
        ):
            identity = singles.tile([P, P], mybir.dt.float16)
            make_identity(nc, identity)

            # Read phase: all reads on the sync ring (4.2MB total fits the
            # DGE queue credit without stalling the engine), hoisted ahead of
            # the writes. g0/g1 ride solo so the pipeline starts early.
            z16_all = z32_pool.tile(
                [P, G_PER_CORE, N_TILES, LATENT_DIM], mybir.dt.float16
            )
            nc.sync.dma_start(out=z16_all[:, 0:1], in_=z_r[:, 0:1])
            nc.sync.dma_start(out=z16_all[:, 1:2], in_=z_r[:, 1:2])
            for a in range(2, G_PER_CORE, 2):
                nc.sync.dma_start(out=z16_all[:, a:a + 2], in_=z_r[:, a:a + 2])

            # Prewarm the ACT sigmoid table during the read phase so the
            # first real sigmoid isn't blocked. The warm op mimics the real
            # ones exactly (fp32 PSUM in, fp8 SBUF out) — a plain fp32->fp32
            # warm loads a DIFFERENT act table (sel=0 vs sel=1), and the real
            # table load then lands on the critical path.
            warm_mm = psum_mm_pool.tile(
                [P, 3 * MAX_NODES], mybir.dt.float32, tag="mm"
            )
            warm_o = singles.tile([P, 8], mybir.dt.float8e4)
            nc.vector.memset(warm_mm[:, 0:8], 0.0)
            nc.scalar.activation(
                out=warm_o, in_=warm_mm[:, 0:8],
                func=mybir.ActivationFunctionType.Sigmoid,
            )

            # Prewarm the PE HAM clock gate: dummy transposes during the read
            # phase flip the PE clock from 1.2 to 2.4 GHz before the first
            # real matmuls arrive. Shares the ps_t tag so no extra PSUM banks.
            warm_ps = psum_t_pool.tile(
                [P, K_TILES, MAX_NODES], mybir.dt.float16, tag="ps_t"
            )
            for _ in range(32):
                nc.tensor.transpose(warm_ps[:, 0, 0:P], identity, identity)

            for g in range(G_PER_CORE):
                z16 = z16_all[:, g]

                # Transpose to zT[p=d % 128, kt, n] (fp16, 1 cycle/row on PE).
                # All 8 transposes of one graph land in ONE psum bank; one DVE
                # copy moves them out.
                zT = zt_pool.tile([P, K_TILES, MAX_NODES], mybir.dt.float16)
                ps_t = psum_t_pool.tile(
                    [P, K_TILES, MAX_NODES], mybir.dt.float16, tag="ps_t"
                )
                for kt in range(K_TILES):
                    for t in range(N_TILES):
                        nc.tensor.transpose(
                            ps_t[:, kt, t * P:(t + 1) * P],
                            z16[:, t, kt * P:(kt + 1) * P],
                            identity,
                        )
                nc.vector.tensor_copy(
                    out=zT.rearrange("p k n -> p (k n)"),
                    in_=ps_t.rearrange("p k n -> p (k n)"),
                )

                # Upper-triangle matmuls into one packed 3-bank PSUM tile:
                # row-block m computes columns [128m, 512), K=256 via 2
                # accumulating matmuls.
                mm = psum_mm_pool.tile(
                    [P, 3 * MAX_NODES], mybir.dt.float32, tag="mm"
                )
                for m, off in SEGS:
                    cs = m * P
                    w = MAX_NODES - cs
                    for kt in range(K_TILES):
                        nc.tensor.matmul(
                            mm[:, off:off + w],
                            lhsT=zT[:, kt, cs:cs + P],
                            rhs=zT[:, kt, cs:MAX_NODES],
                            start=(kt == 0),
                            stop=(kt == K_TILES - 1),
                        )

                # One sigmoid per graph over all packed columns, fp8 out.
                o_t = out_pool.tile([P, PACK_W], mybir.dt.float8e4)
                nc.scalar.activation(
                    out=o_t,
                    in_=mm[:, 0:PACK_W],
                    func=mybir.ActivationFunctionType.Sigmoid,
                )
                nc.sync.dma_start(out=out_t[g], in_=o_t)

    nc.compile()
    return nc


def _get_nc():
    global _NC
    if _NC is None:
        _NC = _build_bass()
    return _NC


def _unpack_packed_triangle(packed):
    """packed [G, 128, 1280] float -> full [G, 512, 512] fp32 (mirrored)."""
    G = packed.shape[0]
    out = np.empty((G, MAX_NODES, MAX_NODES), np.float32)
    out[:, 0:128, :] = packed[:, :, 0:512]
    out[:, 384:512, 384:512] = packed[:, :, 512:640]
    out[:, 128:256, 128:512] = packed[:, :, 640:1024]
    out[:, 256:384, 256:512] = packed[:, :, 1024:1280]
    for mr in range(1, 4):
        for ncl in range(mr):
            out[:, 128 * mr:128 * (mr + 1), 128 * ncl:128 * (ncl + 1)] = (
                out[:, 128 * ncl:128 * (ncl + 1), 128 * mr:128 * (mr + 1)]
                .swapaxes(1, 2)
            )
    return out


def kernel(z, batch, num_graphs, max_nodes):
    global _last_results
    z = np.ascontiguousarray(np.asarray(z), dtype=np.float32)
    batch = np.asarray(batch)
    G = int(num_graphs)
    N = int(max_nodes)
    n_total, d = z.shape
    assert (G, N, d, n_total) == (NUM_GRAPHS, MAX_NODES, LATENT_DIM,
                                  NUM_GRAPHS * MAX_NODES), "hardcoded shapes"

    # Fast path: every graph has exactly max_nodes contiguous nodes.
    expected_batch = (np.arange(n_total) // N).astype(batch.dtype)
    dense = np.array_equal(batch, expected_batch)
    if dense:
        z_full = z
        mask2d = None
    else:
        # General ragged path: scatter into zero-padded [G, N, d] on host,
        # run the same device kernel, then zero out masked positions.
        counts = np.bincount(batch, minlength=G)
        starts = np.concatenate([[0], np.cumsum(counts)[:-1]])
        pos = np.arange(n_total) - starts[batch]
        z_pad = np.zeros((G, N, d), np.float32)
        valid = np.zeros((G, N), bool)
        z_pad[batch, pos] = z
        valid[batch, pos] = True
        z_full = z_pad.reshape(G * N, d)
        mask2d = valid[:, :, None] & valid[:, None, :]

    nc = _get_nc()
    rows = G_PER_CORE * MAX_NODES
    z16_full = z_full.astype(np.float16)
    in_maps = [
        {"z": z16_full[c * rows:(c + 1) * rows]} for c in range(N_CORES)
    ]
    _last_results = run_bass_kernel_spmd(
        nc, in_maps, core_ids=list(range(N_CORES))
    )
    packed = np.concatenate(
        [np.asarray(r["out"]).astype(np.float32) for r in _last_results.results],
        axis=0,
    )  # [128, 128, 1280]
    out = _unpack_packed_triangle(packed)

    if mask2d is not None:
        out = np.where(mask2d, out, np.float32(0.0))
    return out
